# revision 12
# baseline (speedup 1.0000x reference)
"""BERT self-attention (BS=4, SEQ=2048, HID=768, NH=12) on 8 NeuronCores.

Sharding: core c -> batch b = c//2, head-group g = c%2 (6 heads each).

v3 design (vs v1 baseline at 573us, v2 at 337us):
  * Softmax denominator comes free from the ctx matmul: V is stored per
    head as 65 columns (64 V dims + the 0/1 mask column), so ctx PSUM
    row 64 accumulates sum_k m_k * P[k,q].  No denominator matmuls.
  * Scores for the head pair are packed side by side in one
    [128k, 2*512q] PSUM tile -> ONE exp per kb iteration (ACT engine is
    the throughput floor at ~1.1us/iter; the PE must never outpace it
    without filler or it drops out of max pstate).
  * The PE is kept continuously busy (pstate ramp to 2.4 GHz) by
    interleaving QKV projection matmuls as filler inside the attention
    sweeps; V k-blocks are produced just in time during sweep (0,0),
    Q^T chunks just in time for their (j,qc) sweep.
  * QK PSUM drains on DVE fused with the bias add; V bias via an
    appended ones-row on X^T (contraction 769).
  * 1/denom via DVE reciprocal_approx_fast on an SBUF-staged row
    (the custom op drops partition offsets, so stage to partition 0),
    broadcast across partitions by the idle GPSIMD engine.

PSUM (8 banks): scores 2x[128,1024] (4) + ctx 3x[65,512] (3) +
projection/V filler 1x[128,512] (1).
"""

import numpy as np

import concourse.bass as bass
import concourse.tile as tile
from concourse import bacc
from concourse import mybir
from concourse.bass_utils import run_bass_kernel_spmd

F32 = mybir.dt.float32
F16 = mybir.dt.float16
DT_MM = F16
DT_NP = np.float16

BS, SEQ, HID, NH, HD = 4, 2048, 768, 12, 64
NCORES = 8
HPC = 6          # heads per core
FCH = 6          # 128-row chunks of the 768 contraction dim
DSH = HPC * HD   # 384 output features per core
QC = 4           # q chunks of 512
KB = 16          # k blocks of 128

# (j, qc) -> {iter: (kind, j, qc)} projection chunks run as PE filler
# during that attention sweep.  Deadlines: qt[j][:,qc] before sweep
# (j,qc); kt[j] fully before sweep (j,0).  Iters >= 6 so the previous
# sweep's drain has released the filler PSUM slot.
FILL = {
    (0, 0): {14: ("q", 0, 1)},
    (0, 1): {6: ("q", 0, 2), 10: ("k", 1, 0), 13: ("k", 1, 1)},
    (0, 2): {6: ("q", 0, 3), 10: ("k", 1, 2), 13: ("k", 1, 3)},
    (0, 3): {8: ("q", 1, 0)},
    (1, 0): {6: ("q", 1, 1), 10: ("k", 2, 0), 13: ("k", 2, 1)},
    (1, 1): {6: ("q", 1, 2), 10: ("k", 2, 2), 13: ("k", 2, 3)},
    (1, 2): {8: ("q", 1, 3)},
    (1, 3): {8: ("q", 2, 0)},
    (2, 0): {8: ("q", 2, 1)},
    (2, 1): {8: ("q", 2, 2)},
    (2, 2): {8: ("q", 2, 3)},
    (2, 3): {},
}


def _body(tc, xt_d, wq_d, wk_d, wv_d, mt_d, qkb_d, ot_d):
    nc = tc.nc
    Exp = mybir.ActivationFunctionType.Exp

    with tc.tile_pool(name="persist", bufs=1) as persist:
        # Warm the exp table ASAP (overlaps the input DMAs).
        dummy = persist.tile([1, 1], F32, tag="dummy")
        nc.vector.memset(dummy, 0.0)
        nc.scalar.activation(out=dummy, in_=dummy, func=Exp)

        mtile = persist.tile([128, KB], DT_MM, tag="mtile")
        nc.sync.dma_start(out=mtile, in_=mt_d[:, :])
        mtf = persist.tile([128, KB], F32, tag="mtf")
        nc.vector.tensor_copy(out=mtf, in_=mtile)
        qkb = persist.tile([128, 6], F32, tag="qkb")
        nc.sync.dma_start(out=qkb, in_=qkb_d[:, :])

        # Input / weight tiles.  DMA order tracks first use: the K j0
        # projection (gates the first scores) streams x/wk chunk pairs,
        # then wq (Q j0 chunk 0), then wv (V JIT in sweep (0,0)).
        xts, wqs, wks, wvs = [], [], [], []
        for f in range(FCH):
            xts.append(persist.tile([128, SEQ], DT_MM, tag=f"x{f}", name=f"x{f}"))
            wqs.append(persist.tile([128, DSH], DT_MM, tag=f"wq{f}", name=f"wq{f}"))
            wks.append(persist.tile([128, DSH], DT_MM, tag=f"wk{f}", name=f"wk{f}"))
            wvs.append(persist.tile([128, DSH], DT_MM, tag=f"wv{f}", name=f"wv{f}"))
        for f in range(FCH):
            nc.sync.dma_start(out=xts[f], in_=xt_d[f * 128:(f + 1) * 128, :])
            nc.sync.dma_start(out=wks[f], in_=wk_d[f * 128:(f + 1) * 128, :])
        for f in range(FCH):
            nc.sync.dma_start(out=wqs[f], in_=wq_d[f * 128:(f + 1) * 128, :])
        for f in range(FCH):
            nc.sync.dma_start(out=wvs[f], in_=wv_d[f * 128:(f + 1) * 128, :])
        wvb = persist.tile([1, DSH], DT_MM, tag="wvb")
        nc.sync.dma_start(out=wvb, in_=wv_d[768:769, :])
        xt1 = persist.tile([1, SEQ], DT_MM, tag="x6")
        nc.sync.dma_start(out=xt1, in_=xt_d[768:769, :])

        # Q^T/K^T [384, 2048] per head pair j as [128, 2048] tiles.
        qt = [persist.tile([128, SEQ], DT_MM, tag=f"qt{j}", name=f"qt{j}")
              for j in range(3)]
        kt = [persist.tile([128, SEQ], DT_MM, tag=f"kt{j}", name=f"kt{j}")
              for j in range(3)]
        # V with per-head mask column: [k=128, kb, head, 64 V dims + m].
        vt = persist.tile([128, KB, HPC, HD + 1], DT_MM, tag="vt")
        for h in range(HPC):
            nc.vector.tensor_copy(out=vt[:, :, h, HD], in_=mtile)
        ostage = [persist.tile([64, SEQ], F32, tag=f"os{h}", name=f"os{h}")
                  for h in range(HPC)]

        with tc.tile_pool(name="sp", bufs=2, space="PSUM") as sp, \
             tc.tile_pool(name="cp", bufs=3, space="PSUM") as cp, \
             tc.tile_pool(name="fp", bufs=1, space="PSUM") as fp, \
             tc.tile_pool(name="pp", bufs=4) as pp, \
             tc.tile_pool(name="rdp", bufs=2) as rdp:

            def proj_chunk(kind, j, qc):
                """Q or K projection chunk -> qt/kt[j][:, qc*512:...],
                bias folded into the DVE drain."""
                ps = fp.tile([128, 512], F32, tag="f", name="fq")
                qs = slice(qc * 512, (qc + 1) * 512)
                ws = wqs if kind == "q" else wks
                for f in range(FCH):
                    nc.tensor.matmul(ps, lhsT=ws[f][:, j * 128:(j + 1) * 128],
                                     rhs=xts[f][:, qs],
                                     start=(f == 0), stop=(f == FCH - 1))
                dst = (qt if kind == "q" else kt)[j]
                bcol = j if kind == "q" else 3 + j
                nc.vector.tensor_scalar_add(out=dst[:, qs], in0=ps,
                                            scalar1=qkb[:, bcol:bcol + 1])

            def v_chunk(kb):
                """V k-block kb -> vt[:, kb, :, 0:64], mask-scaled rows."""
                ps = fp.tile([128, DSH], F32, tag="f", name="fv")
                ks = slice(kb * 128, (kb + 1) * 128)
                for f in range(FCH):
                    nc.tensor.matmul(ps, lhsT=xts[f][:, ks], rhs=wvs[f],
                                     start=(f == 0), stop=False)
                nc.tensor.matmul(ps, lhsT=xt1[:, ks], rhs=wvb,
                                 start=False, stop=True)
                nc.vector.tensor_scalar_mul(
                    out=vt[:, kb, :, 0:HD], in0=ps,
                    scalar1=mtf[:, kb:kb + 1])

            # Prologue (overlaps the input DMA stream).
            for qc in range(QC):
                proj_chunk("k", 0, qc)
            proj_chunk("q", 0, 0)
            v_chunk(0)
            v_chunk(1)

            for j in range(3):
                heads = (2 * j, 2 * j + 1)
                for qc in range(QC):
                    qs = slice(qc * 512, (qc + 1) * 512)
                    fill_at = FILL[(j, qc)]
                    ctx = [cp.tile([HD + 1, 512], F32, tag="c", name=f"ctx{i}")
                           for i in range(2)]
                    pabs = [None] * KB
                    for kb in range(KB):
                        ks = slice(kb * 128, (kb + 1) * 128)
                        sab = sp.tile([128, 1024], F32, tag="s", name="sab")
                        for i in range(2):
                            rows = slice(64 * i, 64 * (i + 1))
                            nc.tensor.matmul(sab[:, 512 * i:512 * (i + 1)],
                                             lhsT=kt[j][rows, ks],
                                             rhs=qt[j][rows, qs],
                                             start=True, stop=True,
                                             skip_group_check=True)
                        # PE filler between S(kb) and C(kb-1) absorbs the
                        # wait for exp(kb-1) when the PE runs ahead of ACT.
                        if j == 0 and qc == 0 and kb < KB - 2:
                            v_chunk(kb + 2)
                        if kb in fill_at:
                            proj_chunk(*fill_at[kb])
                        if kb >= 1:
                            p = pabs[kb - 1]
                            for i in range(2):
                                nc.tensor.matmul(
                                    ctx[i],
                                    lhsT=vt[:, kb - 1, heads[i], :],
                                    rhs=p[:, 512 * i:512 * (i + 1)],
                                    start=(kb - 1 == 0), stop=False)
                        p = pp.tile([128, 1024], DT_MM, tag="p", name="ptile")
                        nc.scalar.activation(out=p, in_=sab, func=Exp,
                                             scale=0.125)
                        pabs[kb] = p
                    for i in range(2):
                        nc.tensor.matmul(ctx[i],
                                         lhsT=vt[:, KB - 1, heads[i], :],
                                         rhs=pabs[KB - 1][:, 512 * i:512 * (i + 1)],
                                         start=False, stop=True)
                    # drain: out = ctx[0:64] * broadcast(1/ctx[64]).
                    # reciprocal_approx_fast is a custom DVE op that drops
                    # the partition offset of its input, so stage the
                    # denominator row to partition 0 in SBUF first; the
                    # partition broadcast runs on the idle GPSIMD engine.
                    for i in range(2):
                        h = heads[i]
                        dn = rdp.tile([1, 512], F32, tag="dn", name="dn")
                        nc.vector.tensor_copy(out=dn, in_=ctx[i][HD:HD + 1, :])
                        r32 = rdp.tile([1, 512], F32, tag="r32", name="r32")
                        nc.vector.reciprocal_approx_fast(out=r32, in_=dn)
                        pb = rdp.tile([64, 512], F32, tag="pb", name="pb")
                        nc.gpsimd.partition_broadcast(pb, r32)
                        nc.vector.tensor_mul(out=ostage[h][:, qs],
                                             in0=ctx[i][0:HD, :], in1=pb)
                    if qc == QC - 1:
                        for h in heads:
                            nc.sync.dma_start(out=ot_d[h], in_=ostage[h])


def build_nc():
    nc = bacc.Bacc("TRN2")
    xt_d = nc.declare_dram_parameter("xt", [HID + 1, SEQ], DT_MM, isOutput=False)
    wq_d = nc.declare_dram_parameter("wqT", [HID, DSH], DT_MM, isOutput=False)
    wk_d = nc.declare_dram_parameter("wkT", [HID, DSH], DT_MM, isOutput=False)
    wv_d = nc.declare_dram_parameter("wvT", [HID + 1, DSH], DT_MM, isOutput=False)
    mt_d = nc.declare_dram_parameter("mt", [128, KB], DT_MM, isOutput=False)
    qkb_d = nc.declare_dram_parameter("qkb", [128, 6], F32, isOutput=False)
    ot_d = nc.declare_dram_parameter("OT", [HPC, HD, SEQ], F32, isOutput=True)
    with tile.TileContext(nc) as tc:
        _body(tc, xt_d, wq_d, wk_d, wv_d, mt_d, qkb_d, ot_d)
    nc.finalize()
    return nc


_NC_CACHE = None


def _get_nc():
    global _NC_CACHE
    if _NC_CACHE is None:
        _NC_CACHE = build_nc()
    return _NC_CACHE


def make_in_maps(hidden_states, attention_mask, Wq, bq, Wk, bk, Wv, bv):
    in_maps = []
    for c in range(NCORES):
        b, g = c // 2, c % 2
        hs = slice(g * DSH, (g + 1) * DSH)
        xt = np.empty((HID + 1, SEQ), DT_NP)
        xt[:HID] = hidden_states[b].T
        xt[HID] = 1.0
        m = (attention_mask[b, 0, 0] > -1).astype(DT_NP)
        mt = np.ascontiguousarray(m.reshape(KB, 128).T)

        qkb = np.empty((128, 6), np.float32)
        for j in range(3):
            qkb[:, j] = bq[g * DSH + j * 128: g * DSH + (j + 1) * 128]
            qkb[:, 3 + j] = bk[g * DSH + j * 128: g * DSH + (j + 1) * 128]

        def augv(W, bias):
            wa = np.empty((HID + 1, DSH), DT_NP)
            wa[:HID] = W[hs, :].T
            wa[HID] = bias[hs]
            return wa

        in_maps.append({
            "xt": np.ascontiguousarray(xt),
            "wqT": np.ascontiguousarray(Wq[hs, :].T.astype(DT_NP)),
            "wkT": np.ascontiguousarray(Wk[hs, :].T.astype(DT_NP)),
            "wvT": augv(Wv, bv),
            "mt": mt,
            "qkb": qkb,
        })
    return in_maps


def gather_out(results):
    out = np.empty((BS, SEQ, HID), np.float32)
    for c in range(NCORES):
        b, g = c // 2, c % 2
        ot = results[c]["OT"]  # [6, 64, 2048]
        out[b, :, g * DSH:(g + 1) * DSH] = (
            ot.transpose(2, 0, 1).reshape(SEQ, DSH)
        )
    return out


def kernel(hidden_states, attention_mask, Wq, bq, Wk, bk, Wv, bv):
    nc = _get_nc()
    in_maps = make_in_maps(hidden_states, attention_mask,
                           Wq, bq, Wk, bk, Wv, bv)
    res = run_bass_kernel_spmd(nc, in_maps, core_ids=list(range(NCORES)))
    return gather_out(res.results)


# revision 14
# speedup vs baseline: 1.1622x; 1.1622x over previous
"""BERT self-attention (BS=4, SEQ=2048, HID=768, NH=12) on 8 NeuronCores.

Sharding: core c -> batch b = c//2, head-group g = c%2 (6 heads each).

v3 design (vs v1 baseline at 573us, v2 at 337us):
  * Softmax denominator comes free from the ctx matmul: V is stored per
    head as 65 columns (64 V dims + the 0/1 mask column), so ctx PSUM
    row 64 accumulates sum_k m_k * P[k,q].  No denominator matmuls.
  * Scores for the head pair are packed side by side in one
    [128k, 2*512q] PSUM tile -> ONE exp per kb iteration (ACT engine is
    the throughput floor at ~1.1us/iter; the PE must never outpace it
    without filler or it drops out of max pstate).
  * The PE is kept continuously busy (pstate ramp to 2.4 GHz) by
    interleaving QKV projection matmuls as filler inside the attention
    sweeps; V k-blocks are produced just in time during sweep (0,0),
    Q^T chunks just in time for their (j,qc) sweep.
  * QK PSUM drains on DVE fused with the bias add; V bias via an
    appended ones-row on X^T (contraction 769).
  * 1/denom via DVE reciprocal_approx_fast on an SBUF-staged row
    (the custom op drops partition offsets, so stage to partition 0),
    broadcast across partitions by the idle GPSIMD engine.

PSUM (8 banks): scores 2x[128,1024] (4) + ctx 3x[65,512] (3) +
projection/V filler 1x[128,512] (1).
"""

import numpy as np

import concourse.bass as bass
import concourse.tile as tile
from concourse import bacc
from concourse import mybir
from concourse.bass_utils import run_bass_kernel_spmd

F32 = mybir.dt.float32
F16 = mybir.dt.float16
DT_MM = F16
DT_NP = np.float16

BS, SEQ, HID, NH, HD = 4, 2048, 768, 12, 64
NCORES = 8
HPC = 6          # heads per core
FCH = 6          # 128-row chunks of the 768 contraction dim
DSH = HPC * HD   # 384 output features per core
QC = 4           # q chunks of 512
KB = 16          # k blocks of 128

# (j, qc) -> {iter: (kind, j, qc)} projection chunks run as PE filler
# during that attention sweep.  Deadlines: qt[j][:,qc] before sweep
# (j,qc); kt[j] fully before sweep (j,0).  Iters >= 6 so the previous
# sweep's drain has released the filler PSUM slot.
FILL = {
    (0, 0): {14: ("q", 0, 1)},
    (0, 1): {6: ("q", 0, 2), 10: ("k", 1, 0), 13: ("k", 1, 1)},
    (0, 2): {6: ("q", 0, 3), 10: ("k", 1, 2), 13: ("k", 1, 3)},
    (0, 3): {8: ("q", 1, 0)},
    (1, 0): {6: ("q", 1, 1), 10: ("k", 2, 0), 13: ("k", 2, 1)},
    (1, 1): {6: ("q", 1, 2), 10: ("k", 2, 2), 13: ("k", 2, 3)},
    (1, 2): {8: ("q", 1, 3)},
    (1, 3): {8: ("q", 2, 0)},
    (2, 0): {8: ("q", 2, 1)},
    (2, 1): {8: ("q", 2, 2)},
    (2, 2): {8: ("q", 2, 3)},
    (2, 3): {},
}


def _body(tc, xt_d, wq_d, wk_d, wv_d, mt_d, qkb_d, ot_d):
    nc = tc.nc
    Exp = mybir.ActivationFunctionType.Exp

    with tc.tile_pool(name="persist", bufs=1) as persist:
        # Warm the exp table ASAP (overlaps the input DMAs).
        dummy = persist.tile([1, 1], F32, tag="dummy")
        nc.vector.memset(dummy, 0.0)
        nc.scalar.activation(out=dummy, in_=dummy, func=Exp)

        mtile = persist.tile([128, KB], DT_MM, tag="mtile")
        nc.sync.dma_start(out=mtile, in_=mt_d[:, :])
        mtf = persist.tile([128, KB], F32, tag="mtf")
        nc.vector.tensor_copy(out=mtf, in_=mtile)
        qkb = persist.tile([128, 6], F32, tag="qkb")
        nc.sync.dma_start(out=qkb, in_=qkb_d[:, :])

        # Input / weight tiles.  DMA order tracks first use: the K j0
        # projection (gates the first scores) streams x/wk chunk pairs,
        # then wq (Q j0 chunk 0), then wv (V JIT in sweep (0,0)).
        xts, wqs, wks, wvs = [], [], [], []
        for f in range(FCH):
            xts.append(persist.tile([128, SEQ], DT_MM, tag=f"x{f}", name=f"x{f}"))
            wqs.append(persist.tile([128, DSH], DT_MM, tag=f"wq{f}", name=f"wq{f}"))
            wks.append(persist.tile([128, DSH], DT_MM, tag=f"wk{f}", name=f"wk{f}"))
            wvs.append(persist.tile([128, DSH], DT_MM, tag=f"wv{f}", name=f"wv{f}"))
        for f in range(FCH):
            nc.sync.dma_start(out=xts[f], in_=xt_d[f * 128:(f + 1) * 128, :])
            nc.sync.dma_start(out=wks[f], in_=wk_d[f * 128:(f + 1) * 128, :])
        for f in range(FCH):
            nc.sync.dma_start(out=wqs[f], in_=wq_d[f * 128:(f + 1) * 128, :])
        for f in range(FCH):
            nc.sync.dma_start(out=wvs[f], in_=wv_d[f * 128:(f + 1) * 128, :])
        wvb = persist.tile([1, DSH], DT_MM, tag="wvb")
        nc.sync.dma_start(out=wvb, in_=wv_d[768:769, :])
        xt1 = persist.tile([1, SEQ], DT_MM, tag="x6")
        nc.sync.dma_start(out=xt1, in_=xt_d[768:769, :])

        # Q^T/K^T [384, 2048] per head pair j as [128, 2048] tiles.
        qt = [persist.tile([128, SEQ], DT_MM, tag=f"qt{j}", name=f"qt{j}")
              for j in range(3)]
        kt = [persist.tile([128, SEQ], DT_MM, tag=f"kt{j}", name=f"kt{j}")
              for j in range(3)]
        # V with per-head mask column: [k=128, kb, head, 64 V dims + m].
        vt = persist.tile([128, KB, HPC, HD + 1], DT_MM, tag="vt")
        for h in range(HPC):
            nc.vector.tensor_copy(out=vt[:, :, h, HD], in_=mtile)
        ostage = [persist.tile([64, SEQ], F32, tag=f"os{h}", name=f"os{h}")
                  for h in range(HPC)]

        def make_proj(fpool):
            def proj_chunk(kind, j, qc):
                """Q or K projection chunk -> qt/kt[j][:, qc*512:...],
                bias folded into the DVE drain."""
                ps = fpool.tile([128, 512], F32, tag="f", name="fq")
                qs = slice(qc * 512, (qc + 1) * 512)
                ws = wqs if kind == "q" else wks
                for f in range(FCH):
                    nc.tensor.matmul(ps, lhsT=ws[f][:, j * 128:(j + 1) * 128],
                                     rhs=xts[f][:, qs],
                                     start=(f == 0), stop=(f == FCH - 1))
                dst = (qt if kind == "q" else kt)[j]
                bcol = j if kind == "q" else 3 + j
                nc.vector.tensor_scalar_add(out=dst[:, qs], in0=ps,
                                            scalar1=qkb[:, bcol:bcol + 1])

            def v_chunk(kb):
                """V k-block kb -> vt[:, kb, :, 0:64], mask-scaled rows."""
                ps = fpool.tile([128, DSH], F32, tag="f", name="fv")
                ks = slice(kb * 128, (kb + 1) * 128)
                for f in range(FCH):
                    nc.tensor.matmul(ps, lhsT=xts[f][:, ks], rhs=wvs[f],
                                     start=(f == 0), stop=False)
                nc.tensor.matmul(ps, lhsT=xt1[:, ks], rhs=wvb,
                                 start=False, stop=True)
                nc.vector.tensor_scalar_mul(
                    out=vt[:, kb, :, 0:HD], in0=ps,
                    scalar1=mtf[:, kb:kb + 1])

            return proj_chunk, v_chunk

        # Prologue (overlaps the input DMA stream) in its own multi-buffer
        # PSUM pool so chunks pipeline at PE speed, not drain-ring speed.
        with tc.tile_pool(name="pre", bufs=3, space="PSUM") as pre:
            proj_chunk, v_chunk = make_proj(pre)
            for qc in range(QC):
                proj_chunk("k", 0, qc)
            proj_chunk("q", 0, 0)
            v_chunk(0)
            v_chunk(1)

        with tc.tile_pool(name="sp", bufs=2, space="PSUM") as sp, \
             tc.tile_pool(name="cp", bufs=3, space="PSUM") as cp, \
             tc.tile_pool(name="fp", bufs=1, space="PSUM") as fp, \
             tc.tile_pool(name="pp", bufs=4) as pp, \
             tc.tile_pool(name="rdp", bufs=2) as rdp:
            proj_chunk, v_chunk = make_proj(fp)

            for j in range(3):
                heads = (2 * j, 2 * j + 1)
                for qc in range(QC):
                    qs = slice(qc * 512, (qc + 1) * 512)
                    fill_at = FILL[(j, qc)]
                    ctx = [cp.tile([HD + 1, 512], F32, tag="c", name=f"ctx{i}")
                           for i in range(2)]
                    pabs = [None] * KB
                    for kb in range(KB):
                        ks = slice(kb * 128, (kb + 1) * 128)
                        sab = sp.tile([128, 1024], F32, tag="s", name="sab")
                        for i in range(2):
                            rows = slice(64 * i, 64 * (i + 1))
                            nc.tensor.matmul(sab[:, 512 * i:512 * (i + 1)],
                                             lhsT=kt[j][rows, ks],
                                             rhs=qt[j][rows, qs],
                                             start=True, stop=True,
                                             skip_group_check=True)
                        # PE filler between S(kb) and C(kb-1) absorbs the
                        # wait for exp(kb-1) when the PE runs ahead of ACT.
                        if j == 0 and qc == 0 and kb < KB - 2:
                            v_chunk(kb + 2)
                        if kb in fill_at:
                            proj_chunk(*fill_at[kb])
                        if kb >= 1:
                            p = pabs[kb - 1]
                            for i in range(2):
                                nc.tensor.matmul(
                                    ctx[i],
                                    lhsT=vt[:, kb - 1, heads[i], :],
                                    rhs=p[:, 512 * i:512 * (i + 1)],
                                    start=(kb - 1 == 0), stop=False)
                        p = pp.tile([128, 1024], DT_MM, tag="p", name="ptile")
                        nc.scalar.activation(out=p, in_=sab, func=Exp,
                                             scale=0.125)
                        pabs[kb] = p
                    for i in range(2):
                        nc.tensor.matmul(ctx[i],
                                         lhsT=vt[:, KB - 1, heads[i], :],
                                         rhs=pabs[KB - 1][:, 512 * i:512 * (i + 1)],
                                         start=False, stop=True)
                    # drain: out = ctx[0:64] * broadcast(1/ctx[64]).
                    # reciprocal_approx_fast is a custom DVE op that drops
                    # the partition offset of its input, so stage the
                    # denominator row to partition 0 in SBUF first.  The
                    # broadcast to 64 rows is a tiny ones-matmul (NOT the
                    # GPSIMD partition_broadcast: touching GPSIMD compute
                    # downclocks the whole core ~20%).
                    for i in range(2):
                        h = heads[i]
                        dn = rdp.tile([1, 512], F32, tag="dn", name="dn")
                        nc.vector.tensor_copy(out=dn, in_=ctx[i][HD:HD + 1, :])
                        r32 = rdp.tile([1, 512], F32, tag="r32", name="r32")
                        nc.vector.reciprocal_approx_fast(out=r32, in_=dn)
                        rd = rdp.tile([1, 512], DT_MM, tag="r16", name="rd")
                        nc.vector.tensor_copy(out=rd, in_=r32)
                        bc = fp.tile([64, 512], F32, tag="f", name="bc")
                        nc.tensor.matmul(bc, lhsT=xt1[:, 0:64], rhs=rd,
                                         start=True, stop=True)
                        bcs = rdp.tile([64, 512], F32, tag="bcs", name="bcs")
                        nc.vector.tensor_copy(out=bcs, in_=bc)
                        nc.vector.tensor_mul(out=ostage[h][:, qs],
                                             in0=ctx[i][0:HD, :], in1=bcs)
                    if qc == QC - 1:
                        for h in heads:
                            nc.sync.dma_start(out=ot_d[h], in_=ostage[h])


def build_nc():
    nc = bacc.Bacc("TRN2")
    xt_d = nc.declare_dram_parameter("xt", [HID + 1, SEQ], DT_MM, isOutput=False)
    wq_d = nc.declare_dram_parameter("wqT", [HID, DSH], DT_MM, isOutput=False)
    wk_d = nc.declare_dram_parameter("wkT", [HID, DSH], DT_MM, isOutput=False)
    wv_d = nc.declare_dram_parameter("wvT", [HID + 1, DSH], DT_MM, isOutput=False)
    mt_d = nc.declare_dram_parameter("mt", [128, KB], DT_MM, isOutput=False)
    qkb_d = nc.declare_dram_parameter("qkb", [128, 6], F32, isOutput=False)
    ot_d = nc.declare_dram_parameter("OT", [HPC, HD, SEQ], F32, isOutput=True)
    with tile.TileContext(nc) as tc:
        _body(tc, xt_d, wq_d, wk_d, wv_d, mt_d, qkb_d, ot_d)
    nc.finalize()
    return nc


_NC_CACHE = None


def _get_nc():
    global _NC_CACHE
    if _NC_CACHE is None:
        _NC_CACHE = build_nc()
    return _NC_CACHE


def make_in_maps(hidden_states, attention_mask, Wq, bq, Wk, bk, Wv, bv):
    in_maps = []
    for c in range(NCORES):
        b, g = c // 2, c % 2
        hs = slice(g * DSH, (g + 1) * DSH)
        xt = np.empty((HID + 1, SEQ), DT_NP)
        xt[:HID] = hidden_states[b].T
        xt[HID] = 1.0
        m = (attention_mask[b, 0, 0] > -1).astype(DT_NP)
        mt = np.ascontiguousarray(m.reshape(KB, 128).T)

        qkb = np.empty((128, 6), np.float32)
        for j in range(3):
            qkb[:, j] = bq[g * DSH + j * 128: g * DSH + (j + 1) * 128]
            qkb[:, 3 + j] = bk[g * DSH + j * 128: g * DSH + (j + 1) * 128]

        def augv(W, bias):
            wa = np.empty((HID + 1, DSH), DT_NP)
            wa[:HID] = W[hs, :].T
            wa[HID] = bias[hs]
            return wa

        in_maps.append({
            "xt": np.ascontiguousarray(xt),
            "wqT": np.ascontiguousarray(Wq[hs, :].T.astype(DT_NP)),
            "wkT": np.ascontiguousarray(Wk[hs, :].T.astype(DT_NP)),
            "wvT": augv(Wv, bv),
            "mt": mt,
            "qkb": qkb,
        })
    return in_maps


def gather_out(results):
    out = np.empty((BS, SEQ, HID), np.float32)
    for c in range(NCORES):
        b, g = c // 2, c % 2
        ot = results[c]["OT"]  # [6, 64, 2048]
        out[b, :, g * DSH:(g + 1) * DSH] = (
            ot.transpose(2, 0, 1).reshape(SEQ, DSH)
        )
    return out


def kernel(hidden_states, attention_mask, Wq, bq, Wk, bk, Wv, bv):
    nc = _get_nc()
    in_maps = make_in_maps(hidden_states, attention_mask,
                           Wq, bq, Wk, bk, Wv, bv)
    res = run_bass_kernel_spmd(nc, in_maps, core_ids=list(range(NCORES)))
    return gather_out(res.results)


# revision 15
# speedup vs baseline: 1.1702x; 1.0069x over previous
"""BERT self-attention (BS=4, SEQ=2048, HID=768, NH=12) on 8 NeuronCores.

Sharding: core c -> batch b = c//2, head-group g = c%2 (6 heads each).

v5 design (573us baseline -> 337 -> 295 -> this):
  * Softmax denominator comes free from the ctx matmul: V is stored per
    head as 65 columns (64 V dims + the 0/1 mask column), so ctx PSUM
    row 64 accumulates sum_k m_k * P[k,q].  No denominator matmuls.
  * Scores for the head pair are packed side by side in one
    [128k, 2*512q] PSUM tile -> ONE exp per kb iteration.  The ACT
    engine (~1.11us/exp, 192 exps) is the throughput floor; everything
    else is arranged to keep its stream gapless.
  * The PE is kept continuously busy by interleaving QKV projection
    matmuls as filler inside the attention sweeps; V k-blocks are
    produced just in time during sweep (0,0), Q^T chunks just in time
    for their (j,qc) sweep.
  * Inputs arrive in 8 large DMAs (AP rearrange packs the 128-row
    chunks of X and W into single transfers) so the DMA-bound prologue
    is as short as possible.
  * The (j,qc) drain is software-pipelined across the sweep boundary:
    DVE reciprocal work at iter 0 of the next sweep, the broadcast
    matmul + multiply at iter 1, so the next sweep's scores (and the
    ACT stream) are never blocked behind it.  The final drain uses ACT
    copies (ACT is idle at the tail).
  * 1/denom via DVE reciprocal_approx_fast on an SBUF-staged row (the
    custom op drops partition offsets).  GPSIMD compute is avoided
    entirely: touching it downclocks the whole core ~20%.

PSUM (8 banks): scores 2x[128,1024] (4) + ctx 3x[65,512] (3) +
projection/V/broadcast 1x[128,512] (1); prologue borrows 3 banks
before the attention pools open.
"""

import numpy as np

import concourse.bass as bass
import concourse.tile as tile
from concourse import bacc
from concourse import mybir
from concourse.bass_utils import run_bass_kernel_spmd

F32 = mybir.dt.float32
F16 = mybir.dt.float16
DT_MM = F16
DT_NP = np.float16

BS, SEQ, HID, NH, HD = 4, 2048, 768, 12, 64
NCORES = 8
HPC = 6          # heads per core
FCH = 6          # 128-row chunks of the 768 contraction dim
DSH = HPC * HD   # 384 output features per core
QC = 4           # q chunks of 512
KB = 16          # k blocks of 128

# (j, qc) -> {iter: (kind, j, qc)} projection chunks run as PE filler
# during that attention sweep.  Deadlines: qt[j][:,qc] before sweep
# (j,qc); kt[j] fully before sweep (j,0).  Iters chosen to dodge the
# deferred-drain PSUM slot reuse at iters 0-1.
FILL = {
    (0, 0): {14: ("q", 0, 1)},
    (0, 1): {6: ("q", 0, 2), 10: ("k", 1, 0), 13: ("k", 1, 1)},
    (0, 2): {6: ("q", 0, 3), 10: ("k", 1, 2), 13: ("k", 1, 3)},
    (0, 3): {8: ("q", 1, 0)},
    (1, 0): {6: ("q", 1, 1), 10: ("k", 2, 0), 13: ("k", 2, 1)},
    (1, 1): {6: ("q", 1, 2), 10: ("k", 2, 2), 13: ("k", 2, 3)},
    (1, 2): {8: ("q", 1, 3)},
    (1, 3): {8: ("q", 2, 0)},
    (2, 0): {8: ("q", 2, 1)},
    (2, 1): {8: ("q", 2, 2)},
    (2, 2): {8: ("q", 2, 3)},
    (2, 3): {},
}


def _body(tc, xt_d, wq_d, wk_d, wv_d, msc_d, ot_d):
    nc = tc.nc
    Exp = mybir.ActivationFunctionType.Exp

    with tc.tile_pool(name="persist", bufs=1) as persist:
        # Warm the exp table ASAP (overlaps the input DMAs).
        dummy = persist.tile([1, 1], F32, tag="dummy")
        nc.vector.memset(dummy, 0.0)
        nc.scalar.activation(out=dummy, in_=dummy, func=Exp)

        # mask (f32) + Q/K biases in one small DMA.
        msc = persist.tile([128, KB + 6], F32, tag="msc")
        nc.sync.dma_start(out=msc, in_=msc_d[:, :])
        mtf = msc[:, 0:KB]
        qkb = msc[:, KB:KB + 6]
        mtile = persist.tile([128, KB], DT_MM, tag="mtile")
        nc.vector.tensor_copy(out=mtile, in_=mtf)

        # X^T and weights in 7 large DMAs (order = first use: wk gates
        # the K j0 projection, then X, wq, wv).
        wkt = persist.tile([128, FCH, DSH], DT_MM, tag="wkt")
        nc.sync.dma_start(out=wkt, in_=wk_d.rearrange("(f p) d -> p f d", p=128))
        xta = persist.tile([128, 3, SEQ], DT_MM, tag="xta")
        nc.sync.dma_start(out=xta, in_=xt_d[0:384].rearrange("(f p) q -> p f q", p=128))
        xtb = persist.tile([128, 3, SEQ], DT_MM, tag="xtb")
        nc.sync.dma_start(out=xtb, in_=xt_d[384:768].rearrange("(f p) q -> p f q", p=128))
        wqt = persist.tile([128, FCH, DSH], DT_MM, tag="wqt")
        nc.sync.dma_start(out=wqt, in_=wq_d.rearrange("(f p) d -> p f d", p=128))
        wvt = persist.tile([128, FCH, DSH], DT_MM, tag="wvt")
        nc.sync.dma_start(out=wvt, in_=wv_d[0:768].rearrange("(f p) d -> p f d", p=128))
        wvb = persist.tile([1, DSH], DT_MM, tag="wvb")
        nc.sync.dma_start(out=wvb, in_=wv_d[768:769, :])
        xt1 = persist.tile([1, SEQ], DT_MM, tag="x6")
        nc.sync.dma_start(out=xt1, in_=xt_d[768:769, :])

        def xchunk(f):
            return (xta if f < 3 else xtb)[:, f % 3, :]

        # Q^T/K^T [384, 2048] per head pair j as [128, 2048] tiles.
        qt = [persist.tile([128, SEQ], DT_MM, tag=f"qt{j}", name=f"qt{j}")
              for j in range(3)]
        kt = [persist.tile([128, SEQ], DT_MM, tag=f"kt{j}", name=f"kt{j}")
              for j in range(3)]
        # V with per-head mask column: [k=128, kb, head, 64 V dims + m].
        vt = persist.tile([128, KB, HPC, HD + 1], DT_MM, tag="vt")
        for h in range(HPC):
            nc.vector.tensor_copy(out=vt[:, :, h, HD], in_=mtile)
        ostage = [persist.tile([64, SEQ], F32, tag=f"os{h}", name=f"os{h}")
                  for h in range(HPC)]

        def make_proj(fpool):
            def proj_chunk(kind, j, qc):
                """Q or K projection chunk -> qt/kt[j][:, qc*512:...],
                bias folded into the DVE drain."""
                ps = fpool.tile([128, 512], F32, tag="f", name="fq")
                qs = slice(qc * 512, (qc + 1) * 512)
                wt = wqt if kind == "q" else wkt
                for f in range(FCH):
                    nc.tensor.matmul(ps, lhsT=wt[:, f, j * 128:(j + 1) * 128],
                                     rhs=xchunk(f)[:, qs],
                                     start=(f == 0), stop=(f == FCH - 1))
                dst = (qt if kind == "q" else kt)[j]
                bcol = (0 if kind == "q" else 3) + j
                nc.vector.tensor_scalar_add(out=dst[:, qs], in0=ps,
                                            scalar1=qkb[:, bcol:bcol + 1])

            def v_chunk(kb):
                """V k-block kb -> vt[:, kb, :, 0:64], mask-scaled rows."""
                ps = fpool.tile([128, DSH], F32, tag="f", name="fv")
                ks = slice(kb * 128, (kb + 1) * 128)
                for f in range(FCH):
                    nc.tensor.matmul(ps, lhsT=xchunk(f)[:, ks], rhs=wvt[:, f, :],
                                     start=(f == 0), stop=False)
                nc.tensor.matmul(ps, lhsT=xt1[:, ks], rhs=wvb,
                                 start=False, stop=True)
                nc.vector.tensor_scalar_mul(
                    out=vt[:, kb, :, 0:HD], in0=ps,
                    scalar1=mtf[:, kb:kb + 1])

            return proj_chunk, v_chunk

        # Prologue (overlaps the input DMA stream) in its own multi-buffer
        # PSUM pool so chunks pipeline at PE speed, not drain-ring speed.
        with tc.tile_pool(name="pre", bufs=3, space="PSUM") as pre:
            proj_chunk, v_chunk = make_proj(pre)
            for qc in range(QC):
                proj_chunk("k", 0, qc)
            proj_chunk("q", 0, 0)
            v_chunk(0)
            v_chunk(1)

        with tc.tile_pool(name="sp", bufs=2, space="PSUM") as sp, \
             tc.tile_pool(name="cp", bufs=3, space="PSUM") as cp, \
             tc.tile_pool(name="fp", bufs=1, space="PSUM") as fp, \
             tc.tile_pool(name="pp", bufs=4) as pp, \
             tc.tile_pool(name="rdp", bufs=2) as rdp:
            proj_chunk, v_chunk = make_proj(fp)

            def drain_p1(st):
                """DVE part of the deferred drain: 1/denominator rows."""
                st["rd"] = []
                for i in range(2):
                    dn = rdp.tile([1, 512], F32, tag="dn", name="dn")
                    nc.vector.tensor_copy(out=dn, in_=st["ctx"][i][HD:HD + 1, :])
                    r32 = rdp.tile([1, 512], F32, tag="r32", name="r32")
                    nc.vector.reciprocal_approx_fast(out=r32, in_=dn)
                    rd = rdp.tile([1, 512], DT_MM, tag="r16", name="rd")
                    nc.vector.tensor_copy(out=rd, in_=r32)
                    st["rd"].append(rd)

            def drain_p2(st):
                """PE broadcast + multiply + output DMA."""
                for i in range(2):
                    h = st["heads"][i]
                    bc = fp.tile([64, 512], F32, tag="f", name="bc")
                    nc.tensor.matmul(bc, lhsT=xt1[:, 0:64], rhs=st["rd"][i],
                                     start=True, stop=True)
                    bcs = rdp.tile([64, 512], F32, tag="bcs", name="bcs")
                    nc.vector.tensor_copy(out=bcs, in_=bc)
                    nc.vector.tensor_mul(out=ostage[h][:, st["qs"]],
                                         in0=st["ctx"][i][0:HD, :], in1=bcs)
                    nc.sync.dma_start(out=ot_d[h][:, st["qs"]],
                                      in_=ostage[h][:, st["qs"]])

            pending = None
            for j in range(3):
                heads = (2 * j, 2 * j + 1)
                for qc in range(QC):
                    qs = slice(qc * 512, (qc + 1) * 512)
                    fill_at = FILL[(j, qc)]
                    ctx = [cp.tile([HD + 1, 512], F32, tag="c", name=f"ctx{i}")
                           for i in range(2)]
                    pabs = [None] * KB
                    for kb in range(KB):
                        ks = slice(kb * 128, (kb + 1) * 128)
                        sab = sp.tile([128, 1024], F32, tag="s", name="sab")
                        for i in range(2):
                            rows = slice(64 * i, 64 * (i + 1))
                            nc.tensor.matmul(sab[:, 512 * i:512 * (i + 1)],
                                             lhsT=kt[j][rows, ks],
                                             rhs=qt[j][rows, qs],
                                             start=True, stop=True,
                                             skip_group_check=True)
                        if kb == 0 and pending is not None:
                            drain_p1(pending)
                        if kb == 1 and pending is not None:
                            drain_p2(pending)
                            pending = None
                        # PE filler between S(kb) and C(kb-1) absorbs the
                        # wait for exp(kb-1) when the PE runs ahead of ACT.
                        if j == 0 and qc == 0 and kb < KB - 2:
                            v_chunk(kb + 2)
                        if kb in fill_at:
                            proj_chunk(*fill_at[kb])
                        if kb >= 1:
                            p = pabs[kb - 1]
                            for i in range(2):
                                nc.tensor.matmul(
                                    ctx[i],
                                    lhsT=vt[:, kb - 1, heads[i], :],
                                    rhs=p[:, 512 * i:512 * (i + 1)],
                                    start=(kb - 1 == 0), stop=False)
                        p = pp.tile([128, 1024], DT_MM, tag="p", name="ptile")
                        nc.scalar.activation(out=p, in_=sab, func=Exp,
                                             scale=0.125)
                        pabs[kb] = p
                    for i in range(2):
                        nc.tensor.matmul(ctx[i],
                                         lhsT=vt[:, KB - 1, heads[i], :],
                                         rhs=pabs[KB - 1][:, 512 * i:512 * (i + 1)],
                                         start=False, stop=True)
                    pending = {"ctx": ctx, "heads": heads, "qs": qs}

            # Final drain: ACT is idle at the tail, so the row stage and
            # broadcast stage copies run there instead of DVE.
            st = pending
            for i in range(2):
                h = st["heads"][i]
                dn = rdp.tile([1, 512], F32, tag="dn", name="dn")
                nc.scalar.copy(out=dn, in_=st["ctx"][i][HD:HD + 1, :])
                r32 = rdp.tile([1, 512], F32, tag="r32", name="r32")
                nc.vector.reciprocal_approx_fast(out=r32, in_=dn)
                rd = rdp.tile([1, 512], DT_MM, tag="r16", name="rd")
                nc.vector.tensor_copy(out=rd, in_=r32)
                bc = fp.tile([64, 512], F32, tag="f", name="bc")
                nc.tensor.matmul(bc, lhsT=xt1[:, 0:64], rhs=rd,
                                 start=True, stop=True)
                bcs = rdp.tile([64, 512], F32, tag="bcs", name="bcs")
                nc.scalar.copy(out=bcs, in_=bc)
                nc.vector.tensor_mul(out=ostage[h][:, st["qs"]],
                                     in0=st["ctx"][i][0:HD, :], in1=bcs)
                nc.sync.dma_start(out=ot_d[h][:, st["qs"]],
                                  in_=ostage[h][:, st["qs"]])


def build_nc():
    nc = bacc.Bacc("TRN2")
    xt_d = nc.declare_dram_parameter("xt", [HID + 1, SEQ], DT_MM, isOutput=False)
    wq_d = nc.declare_dram_parameter("wqT", [HID, DSH], DT_MM, isOutput=False)
    wk_d = nc.declare_dram_parameter("wkT", [HID, DSH], DT_MM, isOutput=False)
    wv_d = nc.declare_dram_parameter("wvT", [HID + 1, DSH], DT_MM, isOutput=False)
    msc_d = nc.declare_dram_parameter("msc", [128, KB + 6], F32, isOutput=False)
    ot_d = nc.declare_dram_parameter("OT", [HPC, HD, SEQ], F32, isOutput=True)
    with tile.TileContext(nc) as tc:
        _body(tc, xt_d, wq_d, wk_d, wv_d, msc_d, ot_d)
    nc.finalize()
    return nc


_NC_CACHE = None


def _get_nc():
    global _NC_CACHE
    if _NC_CACHE is None:
        _NC_CACHE = build_nc()
    return _NC_CACHE


def make_in_maps(hidden_states, attention_mask, Wq, bq, Wk, bk, Wv, bv):
    in_maps = []
    for c in range(NCORES):
        b, g = c // 2, c % 2
        hs = slice(g * DSH, (g + 1) * DSH)
        xt = np.empty((HID + 1, SEQ), DT_NP)
        xt[:HID] = hidden_states[b].T
        xt[HID] = 1.0
        m = (attention_mask[b, 0, 0] > -1).astype(np.float32)

        msc = np.empty((128, KB + 6), np.float32)
        msc[:, 0:KB] = m.reshape(KB, 128).T
        for j in range(3):
            msc[:, KB + j] = bq[g * DSH + j * 128: g * DSH + (j + 1) * 128]
            msc[:, KB + 3 + j] = bk[g * DSH + j * 128: g * DSH + (j + 1) * 128]

        def augv(W, bias):
            wa = np.empty((HID + 1, DSH), DT_NP)
            wa[:HID] = W[hs, :].T
            wa[HID] = bias[hs]
            return wa

        in_maps.append({
            "xt": np.ascontiguousarray(xt),
            "wqT": np.ascontiguousarray(Wq[hs, :].T.astype(DT_NP)),
            "wkT": np.ascontiguousarray(Wk[hs, :].T.astype(DT_NP)),
            "wvT": augv(Wv, bv),
            "msc": msc,
        })
    return in_maps


def gather_out(results):
    out = np.empty((BS, SEQ, HID), np.float32)
    for c in range(NCORES):
        b, g = c // 2, c % 2
        ot = results[c]["OT"]  # [6, 64, 2048]
        out[b, :, g * DSH:(g + 1) * DSH] = (
            ot.transpose(2, 0, 1).reshape(SEQ, DSH)
        )
    return out


def kernel(hidden_states, attention_mask, Wq, bq, Wk, bk, Wv, bv):
    nc = _get_nc()
    in_maps = make_in_maps(hidden_states, attention_mask,
                           Wq, bq, Wk, bk, Wv, bv)
    res = run_bass_kernel_spmd(nc, in_maps, core_ids=list(range(NCORES)))
    return gather_out(res.results)


# revision 16
# speedup vs baseline: 1.1727x; 1.0021x over previous
"""BERT self-attention (BS=4, SEQ=2048, HID=768, NH=12) on 8 NeuronCores.

Sharding: core c -> batch b = c//2, head-group g = c%2 (6 heads each).

v5 design (573us baseline -> 337 -> 295 -> this):
  * Softmax denominator comes free from the ctx matmul: V is stored per
    head as 65 columns (64 V dims + the 0/1 mask column), so ctx PSUM
    row 64 accumulates sum_k m_k * P[k,q].  No denominator matmuls.
  * Scores for the head pair are packed side by side in one
    [128k, 2*512q] PSUM tile -> ONE exp per kb iteration.  The ACT
    engine (~1.11us/exp, 192 exps) is the throughput floor; everything
    else is arranged to keep its stream gapless.
  * The PE is kept continuously busy by interleaving QKV projection
    matmuls as filler inside the attention sweeps; V k-blocks are
    produced just in time during sweep (0,0), Q^T chunks just in time
    for their (j,qc) sweep.
  * Inputs arrive in 8 large DMAs (AP rearrange packs the 128-row
    chunks of X and W into single transfers) so the DMA-bound prologue
    is as short as possible.
  * The (j,qc) drain is software-pipelined across the sweep boundary:
    DVE reciprocal work at iter 0 of the next sweep, the broadcast
    matmul + multiply at iter 1, so the next sweep's scores (and the
    ACT stream) are never blocked behind it.  The final drain uses ACT
    copies (ACT is idle at the tail).
  * 1/denom via DVE reciprocal_approx_fast on an SBUF-staged row (the
    custom op drops partition offsets).  GPSIMD compute is avoided
    entirely: touching it downclocks the whole core ~20%.

PSUM (8 banks): scores 2x[128,1024] (4) + ctx 3x[65,512] (3) +
projection/V/broadcast 1x[128,512] (1); prologue borrows 3 banks
before the attention pools open.
"""

import numpy as np

import concourse.bass as bass
import concourse.tile as tile
from concourse import bacc
from concourse import mybir
from concourse.bass_utils import run_bass_kernel_spmd

F32 = mybir.dt.float32
F16 = mybir.dt.float16
DT_MM = F16
DT_NP = np.float16

BS, SEQ, HID, NH, HD = 4, 2048, 768, 12, 64
NCORES = 8
HPC = 6          # heads per core
FCH = 6          # 128-row chunks of the 768 contraction dim
DSH = HPC * HD   # 384 output features per core
QC = 4           # q chunks of 512
KB = 16          # k blocks of 128

# (j, qc) -> {iter: (kind, j, qc)} projection chunks run as PE filler
# during that attention sweep.  Deadlines: qt[j][:,qc] before sweep
# (j,qc); kt[j] fully before sweep (j,0).  Iters chosen to dodge the
# deferred-drain PSUM slot reuse at iters 0-1.
FILL = {
    (0, 0): {14: ("q", 0, 1)},
    (0, 1): {6: ("q", 0, 2), 10: ("k", 1, 0), 13: ("k", 1, 1)},
    (0, 2): {6: ("q", 0, 3), 10: ("k", 1, 2), 13: ("k", 1, 3)},
    (0, 3): {8: ("q", 1, 0)},
    (1, 0): {6: ("q", 1, 1), 10: ("k", 2, 0), 13: ("k", 2, 1)},
    (1, 1): {6: ("q", 1, 2), 10: ("k", 2, 2), 13: ("k", 2, 3)},
    (1, 2): {8: ("q", 1, 3)},
    (1, 3): {8: ("q", 2, 0)},
    (2, 0): {8: ("q", 2, 1)},
    (2, 1): {8: ("q", 2, 2)},
    (2, 2): {8: ("q", 2, 3)},
    (2, 3): {},
}


def _body(tc, xt_d, wq_d, wk_d, wv_d, msc_d, ot_d):
    nc = tc.nc
    Exp = mybir.ActivationFunctionType.Exp

    with tc.tile_pool(name="persist", bufs=1) as persist:
        # Warm the exp table ASAP (overlaps the input DMAs).
        dummy = persist.tile([1, 1], F32, tag="dummy")
        nc.vector.memset(dummy, 0.0)
        nc.scalar.activation(out=dummy, in_=dummy, func=Exp)

        # mask (f32) + Q/K biases in one small DMA.
        msc = persist.tile([128, KB + 6], F32, tag="msc")
        nc.sync.dma_start(out=msc, in_=msc_d[:, :])
        mtf = msc[:, 0:KB]
        qkb = msc[:, KB:KB + 6]
        mtile = persist.tile([128, KB], DT_MM, tag="mtile")
        nc.vector.tensor_copy(out=mtile, in_=mtf)

        # X^T and weights in 7 large DMAs (order = first use: wk gates
        # the K j0 projection, then X, wq, wv).
        wkt = persist.tile([128, FCH, DSH], DT_MM, tag="wkt")
        nc.sync.dma_start(out=wkt, in_=wk_d.rearrange("(f p) d -> p f d", p=128))
        xta = persist.tile([128, 3, SEQ], DT_MM, tag="xta")
        nc.sync.dma_start(out=xta, in_=xt_d[0:384].rearrange("(f p) q -> p f q", p=128))
        xtb = persist.tile([128, 3, SEQ], DT_MM, tag="xtb")
        nc.sync.dma_start(out=xtb, in_=xt_d[384:768].rearrange("(f p) q -> p f q", p=128))
        wqt = persist.tile([128, FCH, DSH], DT_MM, tag="wqt")
        nc.sync.dma_start(out=wqt, in_=wq_d.rearrange("(f p) d -> p f d", p=128))
        wvt = persist.tile([128, FCH, DSH], DT_MM, tag="wvt")
        nc.sync.dma_start(out=wvt, in_=wv_d[0:768].rearrange("(f p) d -> p f d", p=128))
        wvb = persist.tile([1, DSH], DT_MM, tag="wvb")
        nc.sync.dma_start(out=wvb, in_=wv_d[768:769, :])
        xt1 = persist.tile([1, SEQ], DT_MM, tag="x6")
        nc.sync.dma_start(out=xt1, in_=xt_d[768:769, :])

        def xchunk(f):
            return (xta if f < 3 else xtb)[:, f % 3, :]

        # Q^T/K^T [384, 2048] per head pair j as [128, 2048] tiles.
        qt = [persist.tile([128, SEQ], DT_MM, tag=f"qt{j}", name=f"qt{j}")
              for j in range(3)]
        kt = [persist.tile([128, SEQ], DT_MM, tag=f"kt{j}", name=f"kt{j}")
              for j in range(3)]
        # V with per-head mask column: [k=128, kb, head, 64 V dims + m].
        vt = persist.tile([128, KB, HPC, HD + 1], DT_MM, tag="vt")
        for h in range(HPC):
            nc.vector.tensor_copy(out=vt[:, :, h, HD], in_=mtile)
        ostage = [persist.tile([64, SEQ], F32, tag=f"os{h}", name=f"os{h}")
                  for h in range(HPC)]

        def make_proj(fpool):
            def proj_chunk(kind, j, qc):
                """Q or K projection chunk -> qt/kt[j][:, qc*512:...],
                bias folded into the DVE drain."""
                ps = fpool.tile([128, 512], F32, tag="f", name="fq")
                qs = slice(qc * 512, (qc + 1) * 512)
                wt = wqt if kind == "q" else wkt
                for f in range(FCH):
                    nc.tensor.matmul(ps, lhsT=wt[:, f, j * 128:(j + 1) * 128],
                                     rhs=xchunk(f)[:, qs],
                                     start=(f == 0), stop=(f == FCH - 1))
                dst = (qt if kind == "q" else kt)[j]
                bcol = (0 if kind == "q" else 3) + j
                nc.vector.tensor_scalar_add(out=dst[:, qs], in0=ps,
                                            scalar1=qkb[:, bcol:bcol + 1])

            def v_chunk(kb):
                """V k-block kb -> vt[:, kb, :, 0:64], mask-scaled rows."""
                ps = fpool.tile([128, DSH], F32, tag="f", name="fv")
                ks = slice(kb * 128, (kb + 1) * 128)
                for f in range(FCH):
                    nc.tensor.matmul(ps, lhsT=xchunk(f)[:, ks], rhs=wvt[:, f, :],
                                     start=(f == 0), stop=False)
                nc.tensor.matmul(ps, lhsT=xt1[:, ks], rhs=wvb,
                                 start=False, stop=True)
                nc.vector.tensor_scalar_mul(
                    out=vt[:, kb, :, 0:HD], in0=ps,
                    scalar1=mtf[:, kb:kb + 1])

            return proj_chunk, v_chunk

        # Prologue (overlaps the input DMA stream) in its own multi-buffer
        # PSUM pool so chunks pipeline at PE speed, not drain-ring speed.
        with tc.tile_pool(name="pre", bufs=3, space="PSUM") as pre:
            proj_chunk, v_chunk = make_proj(pre)
            for qc in range(QC):
                proj_chunk("k", 0, qc)
            proj_chunk("q", 0, 0)
            v_chunk(0)
            v_chunk(1)

        with tc.tile_pool(name="sp", bufs=2, space="PSUM") as sp, \
             tc.tile_pool(name="cp", bufs=3, space="PSUM") as cp, \
             tc.tile_pool(name="fp", bufs=1, space="PSUM") as fp, \
             tc.tile_pool(name="pp", bufs=4) as pp, \
             tc.tile_pool(name="rdp", bufs=2) as rdp:
            proj_chunk, v_chunk = make_proj(fp)

            def drain_p1(st, on_act=False):
                """First drain stage: copy ctx out of PSUM (frees the
                accumulator banks for the next-next sweep) and build the
                f16 reciprocal rows.  DVE work; ACT copies at the tail."""
                cpy = nc.scalar.copy if on_act else (
                    lambda out, in_: nc.vector.tensor_copy(out=out, in_=in_))
                st["cs"], st["rd"] = [], []
                for i in range(2):
                    cs = rdp.tile([64, 512], F32, tag=f"cs{i}", name="cs")
                    cpy(out=cs, in_=st["ctx"][i][0:HD, :])
                    dn = rdp.tile([1, 512], F32, tag="dn", name="dn")
                    cpy(out=dn, in_=st["ctx"][i][HD:HD + 1, :])
                    r32 = rdp.tile([1, 512], F32, tag="r32", name="r32")
                    nc.vector.reciprocal_approx_fast(out=r32, in_=dn)
                    rd = rdp.tile([1, 512], DT_MM, tag="r16", name="rd")
                    nc.vector.tensor_copy(out=rd, in_=r32)
                    st["cs"].append(cs)
                    st["rd"].append(rd)

            def drain_p2(st, on_act=False):
                """Second drain stage: broadcast 1/denom to 64 rows (ones
                matmul), multiply, stream the output DMA."""
                cpy = nc.scalar.copy if on_act else (
                    lambda out, in_: nc.vector.tensor_copy(out=out, in_=in_))
                for i in range(2):
                    h = st["heads"][i]
                    bc = fp.tile([64, 512], F32, tag="f", name="bc")
                    nc.tensor.matmul(bc, lhsT=xt1[:, 0:64], rhs=st["rd"][i],
                                     start=True, stop=True)
                    bcs = rdp.tile([64, 512], F32, tag="bcs", name="bcs")
                    cpy(out=bcs, in_=bc)
                    nc.vector.tensor_mul(out=ostage[h][:, st["qs"]],
                                         in0=st["cs"][i], in1=bcs)
                    nc.sync.dma_start(out=ot_d[h][:, st["qs"]],
                                      in_=ostage[h][:, st["qs"]])

            # Uniform software pipeline over all 192 (j,qc,kb) iterations:
            # at iteration t the PE stream is S(t), C(t-1), filler, with
            # exp(t) on ACT — including across sweep boundaries, so the
            # ACT stream never waits for a sweep's trailing ctx matmuls.
            prevc = None     # (ctx pair, heads, kb, pab) for C(t-1)
            pending = None   # previous sweep's drain state
            for j in range(3):
                heads = (2 * j, 2 * j + 1)
                for qc in range(QC):
                    qs = slice(qc * 512, (qc + 1) * 512)
                    fill_at = FILL[(j, qc)]
                    ctx = [cp.tile([HD + 1, 512], F32, tag="c", name=f"ctx{i}")
                           for i in range(2)]
                    for kb in range(KB):
                        ks = slice(kb * 128, (kb + 1) * 128)
                        sab = sp.tile([128, 1024], F32, tag="s", name="sab")
                        for i in range(2):
                            rows = slice(64 * i, 64 * (i + 1))
                            nc.tensor.matmul(sab[:, 512 * i:512 * (i + 1)],
                                             lhsT=kt[j][rows, ks],
                                             rhs=qt[j][rows, qs],
                                             start=True, stop=True,
                                             skip_group_check=True)
                        if prevc is not None:
                            pctx, pheads, pkb, pp_ = prevc
                            for i in range(2):
                                nc.tensor.matmul(
                                    pctx[i],
                                    lhsT=vt[:, pkb, pheads[i], :],
                                    rhs=pp_[:, 512 * i:512 * (i + 1)],
                                    start=(pkb == 0), stop=(pkb == KB - 1))
                        if kb == 0 and pending is not None:
                            drain_p1(pending)
                        if kb == 1 and pending is not None:
                            drain_p2(pending)
                            pending = None
                        if j == 0 and qc == 0 and kb < KB - 2:
                            v_chunk(kb + 2)
                        if kb in fill_at:
                            proj_chunk(*fill_at[kb])
                        p = pp.tile([128, 1024], DT_MM, tag="p", name="ptile")
                        nc.scalar.activation(out=p, in_=sab, func=Exp,
                                             scale=0.125)
                        prevc = (ctx, heads, kb, p)
                    pending = {"ctx": ctx, "heads": heads, "qs": qs}

            # Tail: final ctx pair, then the last drain with its copies on
            # the now-idle ACT engine.
            pctx, pheads, pkb, pp_ = prevc
            for i in range(2):
                nc.tensor.matmul(pctx[i], lhsT=vt[:, pkb, pheads[i], :],
                                 rhs=pp_[:, 512 * i:512 * (i + 1)],
                                 start=False, stop=True)
            drain_p1(pending, on_act=True)
            drain_p2(pending, on_act=True)


def build_nc():
    nc = bacc.Bacc("TRN2")
    xt_d = nc.declare_dram_parameter("xt", [HID + 1, SEQ], DT_MM, isOutput=False)
    wq_d = nc.declare_dram_parameter("wqT", [HID, DSH], DT_MM, isOutput=False)
    wk_d = nc.declare_dram_parameter("wkT", [HID, DSH], DT_MM, isOutput=False)
    wv_d = nc.declare_dram_parameter("wvT", [HID + 1, DSH], DT_MM, isOutput=False)
    msc_d = nc.declare_dram_parameter("msc", [128, KB + 6], F32, isOutput=False)
    ot_d = nc.declare_dram_parameter("OT", [HPC, HD, SEQ], F32, isOutput=True)
    with tile.TileContext(nc) as tc:
        _body(tc, xt_d, wq_d, wk_d, wv_d, msc_d, ot_d)
    nc.finalize()
    return nc


_NC_CACHE = None


def _get_nc():
    global _NC_CACHE
    if _NC_CACHE is None:
        _NC_CACHE = build_nc()
    return _NC_CACHE


def make_in_maps(hidden_states, attention_mask, Wq, bq, Wk, bk, Wv, bv):
    in_maps = []
    for c in range(NCORES):
        b, g = c // 2, c % 2
        hs = slice(g * DSH, (g + 1) * DSH)
        xt = np.empty((HID + 1, SEQ), DT_NP)
        xt[:HID] = hidden_states[b].T
        xt[HID] = 1.0
        m = (attention_mask[b, 0, 0] > -1).astype(np.float32)

        msc = np.empty((128, KB + 6), np.float32)
        msc[:, 0:KB] = m.reshape(KB, 128).T
        for j in range(3):
            msc[:, KB + j] = bq[g * DSH + j * 128: g * DSH + (j + 1) * 128]
            msc[:, KB + 3 + j] = bk[g * DSH + j * 128: g * DSH + (j + 1) * 128]

        def augv(W, bias):
            wa = np.empty((HID + 1, DSH), DT_NP)
            wa[:HID] = W[hs, :].T
            wa[HID] = bias[hs]
            return wa

        in_maps.append({
            "xt": np.ascontiguousarray(xt),
            "wqT": np.ascontiguousarray(Wq[hs, :].T.astype(DT_NP)),
            "wkT": np.ascontiguousarray(Wk[hs, :].T.astype(DT_NP)),
            "wvT": augv(Wv, bv),
            "msc": msc,
        })
    return in_maps


def gather_out(results):
    out = np.empty((BS, SEQ, HID), np.float32)
    for c in range(NCORES):
        b, g = c // 2, c % 2
        ot = results[c]["OT"]  # [6, 64, 2048]
        out[b, :, g * DSH:(g + 1) * DSH] = (
            ot.transpose(2, 0, 1).reshape(SEQ, DSH)
        )
    return out


def kernel(hidden_states, attention_mask, Wq, bq, Wk, bk, Wv, bv):
    nc = _get_nc()
    in_maps = make_in_maps(hidden_states, attention_mask,
                           Wq, bq, Wk, bk, Wv, bv)
    res = run_bass_kernel_spmd(nc, in_maps, core_ids=list(range(NCORES)))
    return gather_out(res.results)


# revision 19
# speedup vs baseline: 1.2010x; 1.0241x over previous
"""BERT self-attention (BS=4, SEQ=2048, HID=768, NH=12) on 8 NeuronCores.

Sharding: core c -> batch b = c//2, head-group g = c%2 (6 heads each).

v5 design (573us baseline -> 337 -> 295 -> this):
  * Softmax denominator comes free from the ctx matmul: V is stored per
    head as 65 columns (64 V dims + the 0/1 mask column), so ctx PSUM
    row 64 accumulates sum_k m_k * P[k,q].  No denominator matmuls.
  * Scores for the head pair are packed side by side in one
    [128k, 2*512q] PSUM tile -> ONE exp per kb iteration.  The ACT
    engine (~1.11us/exp, 192 exps) is the throughput floor; everything
    else is arranged to keep its stream gapless.
  * The PE is kept continuously busy by interleaving QKV projection
    matmuls as filler inside the attention sweeps; V k-blocks are
    produced just in time during sweep (0,0), Q^T chunks just in time
    for their (j,qc) sweep.
  * Inputs arrive in 8 large DMAs (AP rearrange packs the 128-row
    chunks of X and W into single transfers) so the DMA-bound prologue
    is as short as possible.
  * The (j,qc) drain is software-pipelined across the sweep boundary:
    DVE reciprocal work at iter 0 of the next sweep, the broadcast
    matmul + multiply at iter 1, so the next sweep's scores (and the
    ACT stream) are never blocked behind it.  The final drain uses ACT
    copies (ACT is idle at the tail).
  * 1/denom via DVE reciprocal_approx_fast on an SBUF-staged row (the
    custom op drops partition offsets).  GPSIMD compute is avoided
    entirely: touching it downclocks the whole core ~20%.

PSUM (8 banks): scores 2x[128,1024] (4) + ctx 3x[65,512] (3) +
projection/V/broadcast 1x[128,512] (1); prologue borrows 3 banks
before the attention pools open.
"""

import numpy as np

import concourse.bass as bass
import concourse.tile as tile
from concourse import bacc
from concourse import mybir
from concourse.bass_utils import run_bass_kernel_spmd

F32 = mybir.dt.float32
F16 = mybir.dt.float16
DT_MM = F16
DT_NP = np.float16

BS, SEQ, HID, NH, HD = 4, 2048, 768, 12, 64
NCORES = 8
HPC = 6          # heads per core
FCH = 6          # 128-row chunks of the 768 contraction dim
DSH = HPC * HD   # 384 output features per core
QC = 4           # q chunks of 512
KB = 16          # k blocks of 128

# (j, qc) -> {iter: (kind, j, qc)} projection chunks run as PE filler
# during that attention sweep.  Deadlines: qt[j][:,qc] before sweep
# (j,qc); kt[j] fully before sweep (j,0).  Iters chosen to dodge the
# deferred-drain PSUM slot reuse at iters 0-1.
FILL = {
    (0, 0): {14: ("q", 0, 1)},
    (0, 1): {6: ("q", 0, 2), 10: ("k", 1, 0), 13: ("k", 1, 1)},
    (0, 2): {6: ("q", 0, 3), 10: ("k", 1, 2), 13: ("k", 1, 3)},
    (0, 3): {8: ("q", 1, 0)},
    (1, 0): {6: ("q", 1, 1), 10: ("k", 2, 0), 13: ("k", 2, 1)},
    (1, 1): {6: ("q", 1, 2), 10: ("k", 2, 2), 13: ("k", 2, 3)},
    (1, 2): {8: ("q", 1, 3)},
    (1, 3): {8: ("q", 2, 0)},
    (2, 0): {8: ("q", 2, 1)},
    (2, 1): {8: ("q", 2, 2)},
    (2, 2): {8: ("q", 2, 3)},
    (2, 3): {},
}


def _body(tc, xt_d, wq_d, wk_d, wv_d, msc_d, ot_d):
    nc = tc.nc
    Exp = mybir.ActivationFunctionType.Exp

    with tc.tile_pool(name="persist", bufs=1) as persist:
        # Warm the exp table ASAP (overlaps the input DMAs).
        dummy = persist.tile([1, 1], F32, tag="dummy")
        nc.vector.memset(dummy, 0.0)
        nc.scalar.activation(out=dummy, in_=dummy, func=Exp)

        # mask (f32) + Q/K biases in one small DMA.
        msc = persist.tile([128, KB + 6], F32, tag="msc")
        nc.sync.dma_start(out=msc, in_=msc_d[:, :])
        mtf = msc[:, 0:KB]
        qkb = msc[:, KB:KB + 6]
        mtile = persist.tile([128, KB], DT_MM, tag="mtile")
        nc.vector.tensor_copy(out=mtile, in_=mtf)

        # X^T and weights in 7 large DMAs spread across three DGE queues
        # (sync/scalar/vector) so the transfers stream in parallel.
        # Per-queue order tracks first use: wk gates the K j0 projection,
        # then X, wq, wv.
        wkt = persist.tile([128, FCH, DSH], DT_MM, tag="wkt")
        nc.scalar.dma_start(out=wkt, in_=wk_d.rearrange("(f p) d -> p f d", p=128))
        xta = persist.tile([128, 3, SEQ], DT_MM, tag="xta")
        nc.sync.dma_start(out=xta, in_=xt_d[0:384].rearrange("(f p) q -> p f q", p=128))
        wqt = persist.tile([128, FCH, DSH], DT_MM, tag="wqt")
        nc.scalar.dma_start(out=wqt, in_=wq_d.rearrange("(f p) d -> p f d", p=128))
        xtb = persist.tile([128, 3, SEQ], DT_MM, tag="xtb")
        nc.sync.dma_start(out=xtb, in_=xt_d[384:768].rearrange("(f p) q -> p f q", p=128))
        wvt = persist.tile([128, FCH, DSH], DT_MM, tag="wvt")
        nc.scalar.dma_start(out=wvt, in_=wv_d[0:768].rearrange("(f p) d -> p f d", p=128))
        wvb = persist.tile([1, DSH], DT_MM, tag="wvb")
        nc.scalar.dma_start(out=wvb, in_=wv_d[768:769, :])
        xt1 = persist.tile([1, SEQ], DT_MM, tag="x6")
        nc.sync.dma_start(out=xt1, in_=xt_d[768:769, :])

        def xchunk(f):
            return (xta if f < 3 else xtb)[:, f % 3, :]

        # Q^T/K^T [384, 2048] per head pair j as [128, 2048] tiles.
        qt = [persist.tile([128, SEQ], DT_MM, tag=f"qt{j}", name=f"qt{j}")
              for j in range(3)]
        kt = [persist.tile([128, SEQ], DT_MM, tag=f"kt{j}", name=f"kt{j}")
              for j in range(3)]
        # V with per-head mask column: [k=128, kb, head, 64 V dims + m].
        vt = persist.tile([128, KB, HPC, HD + 1], DT_MM, tag="vt")
        for h in range(HPC):
            nc.vector.tensor_copy(out=vt[:, :, h, HD], in_=mtile)
        ostage = [persist.tile([64, SEQ], F32, tag=f"os{h}", name=f"os{h}")
                  for h in range(HPC)]

        def make_proj(fpool):
            def proj_chunk(kind, j, qc):
                """Q or K projection chunk -> qt/kt[j][:, qc*512:...],
                bias folded into the DVE drain."""
                ps = fpool.tile([128, 512], F32, tag="f", name="fq")
                qs = slice(qc * 512, (qc + 1) * 512)
                wt = wqt if kind == "q" else wkt
                for f in range(FCH):
                    nc.tensor.matmul(ps, lhsT=wt[:, f, j * 128:(j + 1) * 128],
                                     rhs=xchunk(f)[:, qs],
                                     start=(f == 0), stop=(f == FCH - 1))
                dst = (qt if kind == "q" else kt)[j]
                bcol = (0 if kind == "q" else 3) + j
                nc.vector.tensor_scalar_add(out=dst[:, qs], in0=ps,
                                            scalar1=qkb[:, bcol:bcol + 1])

            def v_chunk(kb):
                """V k-block kb -> vt[:, kb, :, 0:64], mask-scaled rows."""
                ps = fpool.tile([128, DSH], F32, tag="f", name="fv")
                ks = slice(kb * 128, (kb + 1) * 128)
                for f in range(FCH):
                    nc.tensor.matmul(ps, lhsT=xchunk(f)[:, ks], rhs=wvt[:, f, :],
                                     start=(f == 0), stop=False)
                nc.tensor.matmul(ps, lhsT=xt1[:, ks], rhs=wvb,
                                 start=False, stop=True)
                nc.vector.tensor_scalar_mul(
                    out=vt[:, kb, :, 0:HD], in0=ps,
                    scalar1=mtf[:, kb:kb + 1])

            return proj_chunk, v_chunk

        # Prologue (overlaps the input DMA stream) in its own multi-buffer
        # PSUM pool so chunks pipeline at PE speed, not drain-ring speed.
        with tc.tile_pool(name="pre", bufs=3, space="PSUM") as pre:
            proj_chunk, v_chunk = make_proj(pre)
            for qc in range(QC):
                proj_chunk("k", 0, qc)
            proj_chunk("q", 0, 0)
            v_chunk(0)
            v_chunk(1)

        with tc.tile_pool(name="sp", bufs=2, space="PSUM") as sp, \
             tc.tile_pool(name="cp", bufs=2, space="PSUM") as cp, \
             tc.tile_pool(name="fp", bufs=2, space="PSUM") as fp, \
             tc.tile_pool(name="pp", bufs=6) as pp, \
             tc.tile_pool(name="rdp", bufs=2) as rdp:
            proj_chunk, v_chunk = make_proj(fp)

            def drain_p1(st, on_act=False):
                """First drain stage: copy ctx out of PSUM (frees the
                accumulator banks for the next-next sweep) and build the
                f16 reciprocal rows.  DVE work; ACT copies at the tail."""
                cpy = nc.scalar.copy if on_act else (
                    lambda out, in_: nc.vector.tensor_copy(out=out, in_=in_))
                st["cs"], st["rd"] = [], []
                for i in range(2):
                    cs = rdp.tile([64, 512], F32, tag=f"cs{i}", name="cs")
                    cpy(out=cs, in_=st["ctx"][i][0:HD, :])
                    dn = rdp.tile([1, 512], F32, tag="dn", name="dn")
                    cpy(out=dn, in_=st["ctx"][i][HD:HD + 1, :])
                    r32 = rdp.tile([1, 512], F32, tag="r32", name="r32")
                    nc.vector.reciprocal_approx_fast(out=r32, in_=dn)
                    rd = rdp.tile([1, 512], DT_MM, tag="r16", name="rd")
                    nc.vector.tensor_copy(out=rd, in_=r32)
                    st["cs"].append(cs)
                    st["rd"].append(rd)

            def drain_p2(st, on_act=False):
                """Second drain stage: broadcast 1/denom to 64 rows (ones
                matmul), multiply, stream the output DMA."""
                cpy = nc.scalar.copy if on_act else (
                    lambda out, in_: nc.vector.tensor_copy(out=out, in_=in_))
                for i in range(2):
                    h = st["heads"][i]
                    bc = fp.tile([64, 512], F32, tag="f", name="bc")
                    nc.tensor.matmul(bc, lhsT=xt1[:, 0:64], rhs=st["rd"][i],
                                     start=True, stop=True)
                    bcs = rdp.tile([64, 512], F32, tag="bcs", name="bcs")
                    cpy(out=bcs, in_=bc)
                    nc.vector.tensor_mul(out=ostage[h][:, st["qs"]],
                                         in0=st["cs"][i], in1=bcs)
                    nc.sync.dma_start(out=ot_d[h][:, st["qs"]],
                                      in_=ostage[h][:, st["qs"]])

            # Uniform software pipeline over all 192 (j,qc,kb) iterations:
            # at iteration t the PE stream is S(t), C(t-1), filler, with
            # exp(t) on ACT — including across sweep boundaries, so the
            # ACT stream never waits for a sweep's trailing ctx matmuls.
            prevc = None     # (ctx pair, heads, kb, pab) for C(t-1)
            pending = None   # previous sweep's drain state
            for j in range(3):
                heads = (2 * j, 2 * j + 1)
                for qc in range(QC):
                    qs = slice(qc * 512, (qc + 1) * 512)
                    fill_at = FILL[(j, qc)]
                    ctx = [cp.tile([HD + 1, 512], F32, tag="c", name=f"ctx{i}")
                           for i in range(2)]
                    for kb in range(KB):
                        ks = slice(kb * 128, (kb + 1) * 128)
                        sab = sp.tile([128, 1024], F32, tag="s", name="sab")
                        for i in range(2):
                            rows = slice(64 * i, 64 * (i + 1))
                            nc.tensor.matmul(sab[:, 512 * i:512 * (i + 1)],
                                             lhsT=kt[j][rows, ks],
                                             rhs=qt[j][rows, qs],
                                             start=True, stop=True,
                                             skip_group_check=True)
                        if prevc is not None:
                            pctx, pheads, pkb, pp_ = prevc
                            for i in range(2):
                                nc.tensor.matmul(
                                    pctx[i],
                                    lhsT=vt[:, pkb, pheads[i], :],
                                    rhs=pp_[:, 512 * i:512 * (i + 1)],
                                    start=(pkb == 0), stop=(pkb == KB - 1))
                        if kb == 0 and pending is not None:
                            drain_p1(pending)
                        if kb == 1 and pending is not None:
                            drain_p2(pending)
                            pending = None
                        if j == 0 and qc == 0 and kb < KB - 2:
                            v_chunk(kb + 2)
                        if kb in fill_at:
                            proj_chunk(*fill_at[kb])
                        p = pp.tile([128, 1024], DT_MM, tag="p", name="ptile")
                        nc.scalar.activation(out=p, in_=sab, func=Exp,
                                             scale=0.125)
                        prevc = (ctx, heads, kb, p)
                    pending = {"ctx": ctx, "heads": heads, "qs": qs}

            # Tail: final ctx pair, then the last drain with its copies on
            # the now-idle ACT engine.
            pctx, pheads, pkb, pp_ = prevc
            for i in range(2):
                nc.tensor.matmul(pctx[i], lhsT=vt[:, pkb, pheads[i], :],
                                 rhs=pp_[:, 512 * i:512 * (i + 1)],
                                 start=False, stop=True)
            drain_p1(pending, on_act=True)
            drain_p2(pending, on_act=True)


def build_nc():
    nc = bacc.Bacc("TRN2")
    xt_d = nc.declare_dram_parameter("xt", [HID + 1, SEQ], DT_MM, isOutput=False)
    wq_d = nc.declare_dram_parameter("wqT", [HID, DSH], DT_MM, isOutput=False)
    wk_d = nc.declare_dram_parameter("wkT", [HID, DSH], DT_MM, isOutput=False)
    wv_d = nc.declare_dram_parameter("wvT", [HID + 1, DSH], DT_MM, isOutput=False)
    msc_d = nc.declare_dram_parameter("msc", [128, KB + 6], F32, isOutput=False)
    ot_d = nc.declare_dram_parameter("OT", [HPC, HD, SEQ], F32, isOutput=True)
    with tile.TileContext(nc) as tc:
        _body(tc, xt_d, wq_d, wk_d, wv_d, msc_d, ot_d)
    nc.finalize()
    return nc


_NC_CACHE = None


def _get_nc():
    global _NC_CACHE
    if _NC_CACHE is None:
        _NC_CACHE = build_nc()
    return _NC_CACHE


def make_in_maps(hidden_states, attention_mask, Wq, bq, Wk, bk, Wv, bv):
    in_maps = []
    for c in range(NCORES):
        b, g = c // 2, c % 2
        hs = slice(g * DSH, (g + 1) * DSH)
        xt = np.empty((HID + 1, SEQ), DT_NP)
        xt[:HID] = hidden_states[b].T
        xt[HID] = 1.0
        m = (attention_mask[b, 0, 0] > -1).astype(np.float32)

        msc = np.empty((128, KB + 6), np.float32)
        msc[:, 0:KB] = m.reshape(KB, 128).T
        for j in range(3):
            msc[:, KB + j] = bq[g * DSH + j * 128: g * DSH + (j + 1) * 128]
            msc[:, KB + 3 + j] = bk[g * DSH + j * 128: g * DSH + (j + 1) * 128]

        def augv(W, bias):
            wa = np.empty((HID + 1, DSH), DT_NP)
            wa[:HID] = W[hs, :].T
            wa[HID] = bias[hs]
            return wa

        in_maps.append({
            "xt": np.ascontiguousarray(xt),
            "wqT": np.ascontiguousarray(Wq[hs, :].T.astype(DT_NP)),
            "wkT": np.ascontiguousarray(Wk[hs, :].T.astype(DT_NP)),
            "wvT": augv(Wv, bv),
            "msc": msc,
        })
    return in_maps


def gather_out(results):
    out = np.empty((BS, SEQ, HID), np.float32)
    for c in range(NCORES):
        b, g = c // 2, c % 2
        ot = results[c]["OT"]  # [6, 64, 2048]
        out[b, :, g * DSH:(g + 1) * DSH] = (
            ot.transpose(2, 0, 1).reshape(SEQ, DSH)
        )
    return out


def kernel(hidden_states, attention_mask, Wq, bq, Wk, bk, Wv, bv):
    nc = _get_nc()
    in_maps = make_in_maps(hidden_states, attention_mask,
                           Wq, bq, Wk, bk, Wv, bv)
    res = run_bass_kernel_spmd(nc, in_maps, core_ids=list(range(NCORES)))
    return gather_out(res.results)


# revision 20
# speedup vs baseline: 1.2211x; 1.0167x over previous
"""BERT self-attention (BS=4, SEQ=2048, HID=768, NH=12) on 8 NeuronCores.

Sharding: core c -> batch b = c//2, head-group g = c%2 (6 heads each).

v8 design (573us baseline -> 337 -> 295 -> 292 -> 285 -> this):
  * Softmax denominator comes free from the ctx matmul: V is stored per
    head as 65 columns (64 V dims + the 0/1 mask column), so ctx PSUM
    row 64 accumulates sum_k m_k * P[k,q].  No denominator matmuls.
  * Scores for the head pair are packed side by side in one
    [128k, 2*512q] PSUM tile -> ONE exp per kb iteration; the two
    64-dim score matmuls run concurrently in PE row halves.
  * The PE (~0.5 ns/column streaming) is the global bottleneck, so all
    projection work is interleaved as filler and every spare matmul
    column is trimmed (V bias is applied at the drain on DVE instead of
    16 ones-row matmuls; the broadcast ones vector is memset on chip).
  * Host pre-packs X^T and W^T into partition-major [128, 6*N] layouts:
    input DMAs are fully contiguous (max descriptor efficiency), split
    across the sync (X) and scalar (weights) DGE queues.  Mask and
    biases ride in spare columns of the wk param.
  * The (j,qc) drain is software-pipelined across the sweep boundary
    (reciprocal work at iter 0, broadcast matmul + multiply + output
    DMA at iter 1) and the ctx accumulators are copied out of PSUM
    immediately so the banks recycle; the final drain runs its copies
    on the then-idle ACT engine.
  * 1/denom via DVE reciprocal_approx_fast on an SBUF-staged row (the
    custom op drops partition offsets).  GPSIMD compute is avoided:
    touching it downclocks the whole core ~20%.

PSUM (8 banks): scores 2x[128,1024] (4) + ctx 2x[65,512] (2) +
projection/V/broadcast 2x[128,512] (2); the prologue borrows 3 banks
before the attention pools open.
"""

import numpy as np

import concourse.bass as bass
import concourse.tile as tile
from concourse import bacc
from concourse import mybir
from concourse.bass_utils import run_bass_kernel_spmd

F32 = mybir.dt.float32
F16 = mybir.dt.float16
DT_MM = F16
DT_NP = np.float16

BS, SEQ, HID, NH, HD = 4, 2048, 768, 12, 64
NCORES = 8
HPC = 6          # heads per core
FCH = 6          # 128-row chunks of the 768 contraction dim
DSH = HPC * HD   # 384 output features per core
QC = 4           # q chunks of 512
KB = 16          # k blocks of 128
WKX = FCH * DSH  # start of the mask/bias columns in the wk param

FILL = {
    (0, 0): {14: ("q", 0, 1)},
    (0, 1): {6: ("q", 0, 2), 10: ("k", 1, 0), 13: ("k", 1, 1)},
    (0, 2): {6: ("q", 0, 3), 10: ("k", 1, 2), 13: ("k", 1, 3)},
    (0, 3): {8: ("q", 1, 0)},
    (1, 0): {6: ("q", 1, 1), 10: ("k", 2, 0), 13: ("k", 2, 1)},
    (1, 1): {6: ("q", 1, 2), 10: ("k", 2, 2), 13: ("k", 2, 3)},
    (1, 2): {8: ("q", 1, 3)},
    (1, 3): {8: ("q", 2, 0)},
    (2, 0): {8: ("q", 2, 1)},
    (2, 1): {8: ("q", 2, 2)},
    (2, 2): {8: ("q", 2, 3)},
    (2, 3): {},
}


def _body(tc, xt_d, wq_d, wk_d, wv_d, ot_d):
    nc = tc.nc
    Exp = mybir.ActivationFunctionType.Exp

    with tc.tile_pool(name="persist", bufs=1) as persist:
        # Warm the exp table ASAP (overlaps the input DMAs).
        dummy = persist.tile([1, 1], F32, tag="dummy")
        nc.vector.memset(dummy, 0.0)
        nc.scalar.activation(out=dummy, in_=dummy, func=Exp)

        # Contiguous partition-major input DMAs on two parallel queues.
        wkp = persist.tile([128, WKX + 28], DT_MM, tag="wkp")
        nc.scalar.dma_start(out=wkp, in_=wk_d[:, :])
        xtp = persist.tile([128, FCH, SEQ], DT_MM, tag="xtp")
        nc.sync.dma_start(out=xtp, in_=xt_d[:, :, :])
        wqp = persist.tile([128, FCH, DSH], DT_MM, tag="wqp")
        nc.scalar.dma_start(out=wqp, in_=wq_d[:, :, :])
        wvp = persist.tile([128, FCH, DSH], DT_MM, tag="wvp")
        nc.scalar.dma_start(out=wvp, in_=wv_d[:, :, :])

        # Mask / bias views and casts (wkp cols WKX..): 16 mask, 3 bq,
        # 3 bk, 6 bv (64 rows each).
        mtile = wkp[:, WKX:WKX + KB]
        mtf = persist.tile([128, KB], F32, tag="mtf")
        nc.vector.tensor_copy(out=mtf, in_=mtile)
        qkb = persist.tile([128, 6], F32, tag="qkb")
        nc.vector.tensor_copy(out=qkb, in_=wkp[:, WKX + KB:WKX + KB + 6])
        bvt = persist.tile([128, 6], F32, tag="bvt")
        nc.vector.tensor_copy(out=bvt, in_=wkp[:, WKX + KB + 6:WKX + 28])
        ones64 = persist.tile([1, HD], DT_MM, tag="ones64")
        nc.vector.memset(ones64, 1.0)

        def xchunk(f):
            return xtp[:, f, :]

        qt = [persist.tile([128, SEQ], DT_MM, tag=f"qt{j}", name=f"qt{j}")
              for j in range(3)]
        kt = [persist.tile([128, SEQ], DT_MM, tag=f"kt{j}", name=f"kt{j}")
              for j in range(3)]
        # V with per-head mask column: [k=128, kb, head, 64 V dims + m].
        vt = persist.tile([128, KB, HPC, HD + 1], DT_MM, tag="vt")
        for h in range(HPC):
            nc.vector.tensor_copy(out=vt[:, :, h, HD], in_=mtile)
        ostage = [persist.tile([64, SEQ], F32, tag=f"os{h}", name=f"os{h}")
                  for h in range(HPC)]

        def make_proj(fpool):
            def proj_chunk(kind, j, qc):
                """Q or K projection chunk -> qt/kt[j][:, qc*512:...],
                bias folded into the DVE drain."""
                ps = fpool.tile([128, 512], F32, tag="f", name="fq")
                qs = slice(qc * 512, (qc + 1) * 512)
                wt = wqp if kind == "q" else wkp
                for f in range(FCH):
                    nc.tensor.matmul(
                        ps,
                        lhsT=wt[:, f * DSH + j * 128:f * DSH + (j + 1) * 128]
                        if kind == "k" else wt[:, f, j * 128:(j + 1) * 128],
                        rhs=xchunk(f)[:, qs],
                        start=(f == 0), stop=(f == FCH - 1))
                dst = (qt if kind == "q" else kt)[j]
                bcol = (0 if kind == "q" else 3) + j
                nc.vector.tensor_scalar_add(out=dst[:, qs], in0=ps,
                                            scalar1=qkb[:, bcol:bcol + 1])

            def v_chunk(kb):
                """V k-block kb -> vt[:, kb, :, 0:64], mask-scaled rows.
                bv is applied at the drain (out = ctx/denom + bv)."""
                ps = fpool.tile([128, DSH], F32, tag="f", name="fv")
                ks = slice(kb * 128, (kb + 1) * 128)
                for f in range(FCH):
                    nc.tensor.matmul(ps, lhsT=xchunk(f)[:, ks],
                                     rhs=wvp[:, f, :],
                                     start=(f == 0), stop=(f == FCH - 1))
                nc.vector.tensor_scalar_mul(
                    out=vt[:, kb, :, 0:HD], in0=ps,
                    scalar1=mtf[:, kb:kb + 1])

            return proj_chunk, v_chunk

        # Prologue (overlaps the input DMA stream) in its own multi-buffer
        # PSUM pool so chunks pipeline at PE speed.
        with tc.tile_pool(name="pre", bufs=3, space="PSUM") as pre:
            proj_chunk, v_chunk = make_proj(pre)
            for qc in range(QC):
                proj_chunk("k", 0, qc)
            proj_chunk("q", 0, 0)
            v_chunk(0)
            v_chunk(1)

        with tc.tile_pool(name="sp", bufs=2, space="PSUM") as sp, \
             tc.tile_pool(name="cp", bufs=2, space="PSUM") as cp, \
             tc.tile_pool(name="fp", bufs=2, space="PSUM") as fp, \
             tc.tile_pool(name="pp", bufs=6) as pp, \
             tc.tile_pool(name="rdp", bufs=2) as rdp:
            proj_chunk, v_chunk = make_proj(fp)

            def drain_p1(st, on_act=False):
                """First drain stage: copy ctx out of PSUM (frees the
                accumulator banks) and build the f16 reciprocal rows."""
                cpy = nc.scalar.copy if on_act else (
                    lambda out, in_: nc.vector.tensor_copy(out=out, in_=in_))
                st["cs"], st["rd"] = [], []
                for i in range(2):
                    cs = rdp.tile([64, 512], F32, tag=f"cs{i}", name="cs")
                    cpy(out=cs, in_=st["ctx"][i][0:HD, :])
                    dn = rdp.tile([1, 512], F32, tag="dn", name="dn")
                    cpy(out=dn, in_=st["ctx"][i][HD:HD + 1, :])
                    r32 = rdp.tile([1, 512], F32, tag="r32", name="r32")
                    nc.vector.reciprocal_approx_fast(out=r32, in_=dn)
                    rd = rdp.tile([1, 512], DT_MM, tag="r16", name="rd")
                    nc.vector.tensor_copy(out=rd, in_=r32)
                    st["cs"].append(cs)
                    st["rd"].append(rd)

            def drain_p2(st, on_act=False):
                """Second drain stage: broadcast 1/denom (ones matmul),
                multiply, add bv, stream the output DMA."""
                cpy = nc.scalar.copy if on_act else (
                    lambda out, in_: nc.vector.tensor_copy(out=out, in_=in_))
                for i in range(2):
                    h = st["heads"][i]
                    bc = fp.tile([64, 512], F32, tag="f", name="bc")
                    nc.tensor.matmul(bc, lhsT=ones64, rhs=st["rd"][i],
                                     start=True, stop=True)
                    bcs = rdp.tile([64, 512], F32, tag="bcs", name="bcs")
                    cpy(out=bcs, in_=bc)
                    mo = rdp.tile([64, 512], F32, tag="mo", name="mo")
                    nc.vector.tensor_mul(out=mo, in0=st["cs"][i], in1=bcs)
                    nc.vector.tensor_scalar_add(
                        out=ostage[h][:, st["qs"]], in0=mo,
                        scalar1=bvt[0:HD, h:h + 1])
                    nc.sync.dma_start(out=ot_d[h][:, st["qs"]],
                                      in_=ostage[h][:, st["qs"]])

            # Uniform software pipeline over all 192 (j,qc,kb) iterations:
            # at iteration t the PE stream is S(t), C(t-1), filler —
            # including across sweep boundaries.
            prevc = None
            pending = None
            for j in range(3):
                heads = (2 * j, 2 * j + 1)
                for qc in range(QC):
                    qs = slice(qc * 512, (qc + 1) * 512)
                    fill_at = FILL[(j, qc)]
                    ctx = [cp.tile([HD + 1, 512], F32, tag="c", name=f"ctx{i}")
                           for i in range(2)]
                    for kb in range(KB):
                        ks = slice(kb * 128, (kb + 1) * 128)
                        sab = sp.tile([128, 1024], F32, tag="s", name="sab")
                        for i in range(2):
                            rows = slice(64 * i, 64 * (i + 1))
                            nc.tensor.matmul(sab[:, 512 * i:512 * (i + 1)],
                                             lhsT=kt[j][rows, ks],
                                             rhs=qt[j][rows, qs],
                                             start=True, stop=True,
                                             skip_group_check=True)
                        if prevc is not None:
                            pctx, pheads, pkb, pp_ = prevc
                            for i in range(2):
                                nc.tensor.matmul(
                                    pctx[i],
                                    lhsT=vt[:, pkb, pheads[i], :],
                                    rhs=pp_[:, 512 * i:512 * (i + 1)],
                                    start=(pkb == 0), stop=(pkb == KB - 1))
                        if kb == 0 and pending is not None:
                            drain_p1(pending)
                        if kb == 1 and pending is not None:
                            drain_p2(pending)
                            pending = None
                        if j == 0 and qc == 0 and kb < KB - 2:
                            v_chunk(kb + 2)
                        if kb in fill_at:
                            proj_chunk(*fill_at[kb])
                        p = pp.tile([128, 1024], DT_MM, tag="p", name="ptile")
                        nc.scalar.activation(out=p, in_=sab, func=Exp,
                                             scale=0.125)
                        prevc = (ctx, heads, kb, p)
                    pending = {"ctx": ctx, "heads": heads, "qs": qs}

            # Tail: final ctx pair, then the last drain with its copies on
            # the now-idle ACT engine.
            pctx, pheads, pkb, pp_ = prevc
            for i in range(2):
                nc.tensor.matmul(pctx[i], lhsT=vt[:, pkb, pheads[i], :],
                                 rhs=pp_[:, 512 * i:512 * (i + 1)],
                                 start=False, stop=True)
            drain_p1(pending, on_act=True)
            drain_p2(pending, on_act=True)


def build_nc():
    nc = bacc.Bacc("TRN2")
    xt_d = nc.declare_dram_parameter("xtp", [128, FCH, SEQ], DT_MM, isOutput=False)
    wq_d = nc.declare_dram_parameter("wqp", [128, FCH, DSH], DT_MM, isOutput=False)
    wk_d = nc.declare_dram_parameter("wkp", [128, WKX + 28], DT_MM, isOutput=False)
    wv_d = nc.declare_dram_parameter("wvp", [128, FCH, DSH], DT_MM, isOutput=False)
    ot_d = nc.declare_dram_parameter("OT", [HPC, HD, SEQ], F32, isOutput=True)
    with tile.TileContext(nc) as tc:
        _body(tc, xt_d, wq_d, wk_d, wv_d, ot_d)
    nc.finalize()
    return nc


_NC_CACHE = None


def _get_nc():
    global _NC_CACHE
    if _NC_CACHE is None:
        _NC_CACHE = build_nc()
    return _NC_CACHE


def _pack_pm(m):
    """[768, N] -> partition-major [128, 6*N] (chunk-major free dim)."""
    n = m.shape[1]
    return np.ascontiguousarray(
        m.reshape(FCH, 128, n).transpose(1, 0, 2).reshape(128, FCH * n))


def make_in_maps(hidden_states, attention_mask, Wq, bq, Wk, bk, Wv, bv):
    in_maps = []
    for c in range(NCORES):
        b, g = c // 2, c % 2
        hs = slice(g * DSH, (g + 1) * DSH)
        xtp = _pack_pm(hidden_states[b].T.astype(DT_NP))
        wqp = _pack_pm(Wq[hs, :].T.astype(DT_NP))
        wvp = _pack_pm(Wv[hs, :].T.astype(DT_NP))

        wkp = np.zeros((128, WKX + 28), DT_NP)
        wkp[:, :WKX] = _pack_pm(Wk[hs, :].T.astype(DT_NP))
        m = (attention_mask[b, 0, 0] > -1).astype(DT_NP)
        wkp[:, WKX:WKX + KB] = m.reshape(KB, 128).T
        for j in range(3):
            wkp[:, WKX + KB + j] = bq[g * DSH + j * 128: g * DSH + (j + 1) * 128]
            wkp[:, WKX + KB + 3 + j] = bk[g * DSH + j * 128: g * DSH + (j + 1) * 128]
        for h in range(HPC):
            wkp[0:HD, WKX + KB + 6 + h] = bv[g * DSH + h * HD: g * DSH + (h + 1) * HD]

        in_maps.append({"xtp": xtp, "wqp": wqp, "wkp": wkp, "wvp": wvp})
    return in_maps


def gather_out(results):
    out = np.empty((BS, SEQ, HID), np.float32)
    for c in range(NCORES):
        b, g = c // 2, c % 2
        ot = results[c]["OT"]  # [6, 64, 2048]
        out[b, :, g * DSH:(g + 1) * DSH] = (
            ot.transpose(2, 0, 1).reshape(SEQ, DSH)
        )
    return out


def kernel(hidden_states, attention_mask, Wq, bq, Wk, bk, Wv, bv):
    nc = _get_nc()
    in_maps = make_in_maps(hidden_states, attention_mask,
                           Wq, bq, Wk, bk, Wv, bv)
    res = run_bass_kernel_spmd(nc, in_maps, core_ids=list(range(NCORES)))
    return gather_out(res.results)
